# revision 5
# baseline (speedup 1.0000x reference)
"""Trainium2 Bass kernel for nn_LinearEncoder (gnn_message_passing).

Reference computes, for N=512 nodes with n_in = n_out = 256:
    i, j = triu_indices(N, k=1)
    edges = concat([x[i], x[j]], -1)            # [E, 512]
    h = edges @ W.T + b                         # [E, n_out]
    out[i, j] = h ; out = out + out.T           # [N, N, 256], 0 diagonal

Key identities: with W = [W1 | W2], A = x @ W1.T, B' = x @ W2.T + b,
the full output is symmetric with zero diagonal and, on the upper
triangle (i < j),
    out[i, j, c] = A[i, c] + B'[j, c].
The device therefore only materialises the strict upper triangle; the
host's unshard step places each value at both (i, j) and (j, i) (the
diagonal stays at the scatter-init zero), halving the HBM write stream
versus a full-matrix kernel.

Layout: channels on SBUF partitions (two 128-channel halves), nodes on
the free dimension.  The column tables B'T[c, j] fall straight out of
two K=256 matmuls against the uploaded x.T; the per-row terms A[i, c]
are [128, 1] columns of RS = W1 @ x[rows]entsel.T (host slices the 64
owned rows per core), so each output row segment is a single
tensor-scalar add: out_seg = B'T[:, j0:512] + RS[:, m].

Sharding: core k owns rows i = 32*b + 4*k + v (b in [0,16), v in
[0,4)) — four rows from every 32-row column block, so each core's
upper-triangle rectangles (cols [32b, 512) for block b) have identical
shapes across cores (one SPMD program) and identical total bytes.
Blocks b and 15-b pair into eight [128, 4352] bf16 slabs (1.09 MB
HWDGE DMAs, ~8.9 MB/core total).  Sub-diagonal lanes inside a
rectangle are shipped as garbage and discarded by the host (the mirror
of the transposed upper triangle supplies those entries).
"""

import os
import sys

for _p in ("/opt/trn_rl_repo", "/root/.axon_site/_ro/trn_rl_repo"):
    if os.path.isdir(_p) and _p not in sys.path:
        sys.path.insert(0, _p)

import numpy as np
import ml_dtypes

import concourse.bass as bass
import concourse.bacc as bacc
import concourse.mybir as mybir
import concourse.tile as tile
from concourse.bass_utils import run_bass_kernel_spmd

N = 512
CH = 256          # n_out
NIN = 256         # n_in
NCORES = 8
NB = 16           # column blocks of 32
RPB = 4           # rows per block per core
F32 = mybir.dt.float32
BF16 = mybir.dt.bfloat16
BF16NP = ml_dtypes.bfloat16

SEG = 1088        # per-v slab columns: 2*(w1 + w2), w1 + w2 = 544
SLABW = RPB * SEG  # 4352


def _rows_for_core(k: int) -> np.ndarray:
    """Row m = 4*b + v owns global row 32*b + 4*k + v."""
    b = np.repeat(np.arange(NB), RPB)
    v = np.tile(np.arange(RPB), NB)
    return 32 * b + RPB * k + v


# --------------------------------------------------------------------------
# device program
# --------------------------------------------------------------------------

_PROGRAM = None


def _build_program() -> bass.Bass:
    nc = bacc.Bacc()

    xt = nc.dram_tensor("xt", [NIN, N], BF16, kind="ExternalInput")
    # [W1.T | W2.T]: cols 0:256 -> W1.T (row terms), 256:512 -> W2.T
    w12t = nc.dram_tensor("w12t", [NIN, 2 * CH], BF16, kind="ExternalInput")
    xselt = nc.dram_tensor("xselt", [NIN, 64], BF16, kind="ExternalInput")
    bcol = nc.dram_tensor("bcol", [128, 2], F32, kind="ExternalInput")
    # slab[p]: blocks (p, 15-p); per v in [0,4): [A h0 (w1) | A h1 (w1) |
    # B h0 (w2) | B h1 (w2)] at offset 1088*v, w1 = 512-32p, w2 = 32+32p.
    slab = nc.dram_tensor("slab", [8, 128, SLABW], BF16,
                          kind="ExternalOutput")

    with tile.TileContext(nc) as tc:
        with (
            tc.tile_pool(name="const", bufs=1) as cpool,
            tc.tile_pool(name="psB", bufs=2, space="PSUM") as psB,
            tc.tile_pool(name="psR", bufs=2, space="PSUM") as psR,
            tc.tile_pool(name="slabs", bufs=3) as spool,
        ):
            # ---- input loads: B'T dependencies first ---------------------
            def load(eng, dram, shape, dtype, tag):
                t = cpool.tile(shape, dtype, tag=tag)
                eng.dma_start(out=t[:], in_=dram)
                return t

            xt0 = load(nc.sync, xt[0:128, :], [128, N], BF16, "xt0")
            w0 = load(nc.scalar, w12t[0:128, :], [128, 2 * CH], BF16, "w0")
            xt1 = load(nc.sync, xt[128:256, :], [128, N], BF16, "xt1")
            w1_ = load(nc.scalar, w12t[128:256, :], [128, 2 * CH], BF16, "w1")
            xs0 = load(nc.sync, xselt[0:128, :], [128, 64], BF16, "xs0")
            xs1 = load(nc.sync, xselt[128:256, :], [128, 64], BF16, "xs1")
            bc = load(nc.scalar, bcol[:], [128, 2], F32, "bc")

            mm = nc.tensor.matmul

            # ---- column tables B'T[c, j] = B[j, c] + b[c], two halves ----
            BT = [None, None]
            for h in range(2):
                pb = psB.tile([128, N], F32, tag="pb", name=f"pb{h}")
                lo = 2 * CH - 256 + 128 * h  # W2.T columns 256+128h
                mm(pb[:], w0[:, lo:lo + 128], xt0[:], start=True, stop=False)
                mm(pb[:], w1_[:, lo:lo + 128], xt1[:], start=False, stop=True)
                bt = cpool.tile([128, N], BF16, tag=f"BT{h}")
                if h == 0:
                    nc.vector.tensor_scalar_add(bt[:], pb[:], bc[:, 0:1])
                else:
                    nc.scalar.add(bt[:], pb[:], bc[:, 1:2])
                BT[h] = bt

            # ---- row terms RS[c, m] = A[row_m, c], two halves ------------
            RS = [None, None]
            for h in range(2):
                pr = psR.tile([128, 64], F32, tag="pr", name=f"pr{h}")
                lo = 128 * h  # W1.T columns
                mm(pr[:], w0[:, lo:lo + 128], xs0[:], start=True, stop=False)
                mm(pr[:], w1_[:, lo:lo + 128], xs1[:], start=False, stop=True)
                rs = cpool.tile([128, 64], F32, tag=f"RS{h}")
                if h == 0:
                    nc.vector.tensor_copy(out=rs[:], in_=pr[:])
                else:
                    nc.scalar.copy(out=rs[:], in_=pr[:])
                RS[h] = rs

            # ---- main loop: one slab per block pair ----------------------
            for p in range(8):
                w1 = N - 32 * p          # block p rect width (cols 32p..512)
                w2 = 32 + 32 * p         # block 15-p width (cols 480-32p..512)
                cA = 32 * p              # B'T col offset for block p
                cB = N - w2              # for block 15-p
                S = spool.tile([128, SLABW], BF16, tag="s", name=f"s{p}")
                for v in range(RPB):
                    off = SEG * v
                    mA = RPB * p + v
                    mB = RPB * (15 - p) + v
                    sA0 = S[:, off:off + w1]
                    sA1 = S[:, off + w1:off + 2 * w1]
                    sB0 = S[:, off + 2 * w1:off + 2 * w1 + w2]
                    sB1 = S[:, off + 2 * w1 + w2:off + SEG]
                    nc.vector.tensor_scalar_add(
                        sA0, BT[0][:, cA:N], RS[0][:, mA:mA + 1])
                    nc.scalar.add(sA1, BT[1][:, cA:N], RS[1][:, mA:mA + 1])
                    nc.gpsimd.tensor_scalar_add(
                        sB0, BT[0][:, cB:N], RS[0][:, mB:mB + 1])
                    if p <= 3:
                        nc.gpsimd.tensor_scalar_add(
                            sB1, BT[1][:, cB:N], RS[1][:, mB:mB + 1])
                    else:
                        nc.vector.tensor_scalar_add(
                            sB1, BT[1][:, cB:N], RS[1][:, mB:mB + 1])
                nc.sync.dma_start(out=slab[p], in_=S[:])

    nc.compile()
    return nc


def _program() -> bass.Bass:
    global _PROGRAM
    if _PROGRAM is None:
        _PROGRAM = _build_program()
    return _PROGRAM


# --------------------------------------------------------------------------
# host entry point
# --------------------------------------------------------------------------

def build_in_maps(x, W, b):
    x = np.asarray(x, np.float32)
    W = np.asarray(W, np.float32)
    b = np.asarray(b, np.float32)
    w12 = np.concatenate(
        [np.ascontiguousarray(W[:, :NIN].T),
         np.ascontiguousarray(W[:, NIN:].T)], axis=1)  # [in, 512]
    shared = {
        "xt": np.ascontiguousarray(x.T).astype(BF16NP),
        "w12t": w12.astype(BF16NP),
        "bcol": np.ascontiguousarray(
            np.stack([b[0:128], b[128:256]], axis=1)),
    }
    maps = []
    for k in range(NCORES):
        rows = _rows_for_core(k)
        xsel = np.ascontiguousarray(x[rows].T).astype(BF16NP)
        maps.append(dict(shared, xselt=xsel))
    return maps


def _assemble(results):
    """8 per-core slab dicts -> full [512, 512, 256] f32 output."""
    out = np.zeros((N, N, CH), np.float32)
    ar = np.arange(RPB)
    for k in range(NCORES):
        slab = np.asarray(results[k]["slab"]).astype(np.float32)
        for p in range(8):
            w1 = N - 32 * p
            w2 = 32 + 32 * p
            sp = slab[p].reshape(128, RPB, SEG)
            rowsA = 32 * p + RPB * k + ar
            rowsB = 32 * (15 - p) + RPB * k + ar
            out[rowsA, 32 * p:N, 0:128] = \
                sp[:, :, 0:w1].transpose(1, 2, 0)
            out[rowsA, 32 * p:N, 128:256] = \
                sp[:, :, w1:2 * w1].transpose(1, 2, 0)
            out[rowsB, N - w2:N, 0:128] = \
                sp[:, :, 2 * w1:2 * w1 + w2].transpose(1, 2, 0)
            out[rowsB, N - w2:N, 128:256] = \
                sp[:, :, 2 * w1 + w2:SEG].transpose(1, 2, 0)
    # unshard: keep the strict upper triangle (sub-diagonal rect lanes are
    # garbage), mirror it across the diagonal; diag stays scatter-init 0.
    tril = np.tril_indices(N)
    out[tril] = 0.0
    return out + out.transpose(1, 0, 2)


def kernel(x, W, b):
    nc = _program()
    in_maps = build_in_maps(x, W, b)
    res = run_bass_kernel_spmd(nc, in_maps, core_ids=list(range(NCORES)))
    return _assemble(res.results)


# revision 6
# speedup vs baseline: 2.6644x; 2.6644x over previous
"""Trainium2 Bass kernel for nn_LinearEncoder (gnn_message_passing).

Reference computes, for N=512 nodes with n_in = n_out = 256:
    i, j = triu_indices(N, k=1)
    edges = concat([x[i], x[j]], -1)            # [E, 512]
    h = edges @ W.T + b                         # [E, n_out]
    out[i, j] = h ; out = out + out.T           # [N, N, 256], 0 diagonal

Key identities: with W = [W1 | W2], A = x @ W1.T, B' = x @ W2.T + b,
the full output is symmetric with zero diagonal and, on the upper
triangle (i < j),
    out[i, j, c] = A[i, c] + B'[j, c].
The device therefore only materialises the strict upper triangle; the
host's unshard step places each value at both (i, j) and (j, i) (the
diagonal stays at the scatter-init zero), halving the HBM write stream
versus a full-matrix kernel.

Layout: channels on SBUF partitions (two 128-channel halves), nodes on
the free dimension.  The column tables B'T[c, j] fall straight out of
two K=256 matmuls against the uploaded x.T; the per-row terms A[i, c]
are [128, 1] columns of RS = W1 @ x[rows_k].T (host slices the 64
owned rows per core), so each output row segment is a single
per-partition-scalar add: out_seg = B'T[:, j0:512] + RS[:, m].

Sharding: core k owns rows i = 32*b + 4*k + v (b in [0,16), v in
[0,4)) — four rows from every 32-row column block, so each core's
upper-triangle rectangles (cols [32b, 512) for block b) have identical
shapes across cores (one SPMD program) and identical total bytes.
Blocks b and 15-b pair into eight [128, 4352] bf16 slabs (1.09 MB
HWDGE DMAs, ~8.9 MB/core total).  Sub-diagonal lanes inside a
rectangle are shipped as garbage and discarded by the host (the mirror
of the transposed upper triangle supplies those entries).

Engine assignment is calibrated to the measured TRN2 op costs: DVE
tensor_scalar (4x uop) takes the wide segments, ScalarE ACT-with-bias
the mid ones, and the narrow block-B segments run as one fused
tensor_tensor per half with stride-0/stride-1 broadcast APs over the
four v-rows (GpSimd for p<=3, DVE for p=4,5).  GpSimd tensor_scalar is
never used: it runs ~7x slower than its tensor_tensor and its SBUF
traffic starves concurrent DVE ops.
"""

import os
import sys

for _p in ("/opt/trn_rl_repo", "/root/.axon_site/_ro/trn_rl_repo"):
    if os.path.isdir(_p) and _p not in sys.path:
        sys.path.insert(0, _p)

import numpy as np
import ml_dtypes

import concourse.bass as bass
import concourse.bacc as bacc
import concourse.mybir as mybir
import concourse.tile as tile
from concourse.bass_utils import run_bass_kernel_spmd

N = 512
CH = 256          # n_out
NIN = 256         # n_in
NCORES = 8
NB = 16           # column blocks of 32
RPB = 4           # rows per block per core
F32 = mybir.dt.float32
BF16 = mybir.dt.bfloat16
BF16NP = ml_dtypes.bfloat16

SEG = 1088        # per-v slab columns: 2*(w1 + w2), w1 + w2 = 544
SLABW = RPB * SEG  # 4352


def _rows_for_core(k: int) -> np.ndarray:
    """Row m = 4*b + v owns global row 32*b + 4*k + v."""
    b = np.repeat(np.arange(NB), RPB)
    v = np.tile(np.arange(RPB), NB)
    return 32 * b + RPB * k + v


# --------------------------------------------------------------------------
# device program
# --------------------------------------------------------------------------

_PROGRAM = None


def _build_program() -> bass.Bass:
    nc = bacc.Bacc()
    ADD = mybir.AluOpType.add

    xt = nc.dram_tensor("xt", [NIN, N], BF16, kind="ExternalInput")
    # [W1.T | W2.T]: cols 0:256 -> W1.T (row terms), 256:512 -> W2.T
    w12t = nc.dram_tensor("w12t", [NIN, 2 * CH], BF16, kind="ExternalInput")
    xselt = nc.dram_tensor("xselt", [NIN, 64], BF16, kind="ExternalInput")
    bcol = nc.dram_tensor("bcol", [128, 2], F32, kind="ExternalInput")
    # slab[p]: blocks (p, 15-p); per v in [0,4): [A h0 (w1) | A h1 (w1) |
    # B h0 (w2) | B h1 (w2)] at offset 1088*v, w1 = 512-32p, w2 = 32+32p.
    slab = nc.dram_tensor("slab", [8, 128, SLABW], BF16,
                          kind="ExternalOutput")

    with tile.TileContext(nc) as tc:
        with (
            tc.tile_pool(name="const", bufs=1) as cpool,
            tc.tile_pool(name="psB", bufs=2, space="PSUM") as psB,
            tc.tile_pool(name="psR", bufs=2, space="PSUM") as psR,
            tc.tile_pool(name="slabs", bufs=4) as spool,
        ):
            # ---- input loads: small xsel first, then B'T dependencies ----
            def load(eng, dram, shape, dtype, tag):
                t = cpool.tile(shape, dtype, tag=tag)
                eng.dma_start(out=t[:], in_=dram)
                return t

            xs0 = load(nc.sync, xselt[0:128, :], [128, 64], BF16, "xs0")
            w0 = load(nc.scalar, w12t[0:128, :], [128, 2 * CH], BF16, "w0")
            xt0 = load(nc.sync, xt[0:128, :], [128, N], BF16, "xt0")
            w1_ = load(nc.scalar, w12t[128:256, :], [128, 2 * CH], BF16, "w1")
            xt1 = load(nc.sync, xt[128:256, :], [128, N], BF16, "xt1")
            xs1 = load(nc.scalar, xselt[128:256, :], [128, 64], BF16, "xs1")
            bc = load(nc.sync, bcol[:], [128, 2], F32, "bc")

            mm = nc.tensor.matmul

            # ---- row terms RS[c, m] = A[row_m, c], two halves ------------
            RS = [None, None]    # f32, scalar operands for TS / ACT bias
            RS16 = [None, None]  # bf16, in1 for fused tensor_tensor
            for h in range(2):
                pr = psR.tile([128, 64], F32, tag="pr", name=f"pr{h}")
                lo = 128 * h  # W1.T columns
                mm(pr[:], w0[:, lo:lo + 128], xs0[:], start=True, stop=False)
                mm(pr[:], w1_[:, lo:lo + 128], xs1[:], start=False, stop=True)
                rs = cpool.tile([128, 64], F32, tag=f"RS{h}")
                rs16 = cpool.tile([128, 64], BF16, tag=f"RS16{h}")
                nc.vector.tensor_copy(out=rs[:], in_=pr[:])
                nc.scalar.copy(out=rs16[:], in_=pr[:])
                RS[h] = rs
                RS16[h] = rs16

            # ---- column tables B'T[c, j] = B[j, c] + b[c], two halves ----
            BT = [None, None]
            for h in range(2):
                pb = psB.tile([128, N], F32, tag="pb", name=f"pb{h}")
                lo = 256 + 128 * h  # W2.T columns
                mm(pb[:], w0[:, lo:lo + 128], xt0[:], start=True, stop=False)
                mm(pb[:], w1_[:, lo:lo + 128], xt1[:], start=False, stop=True)
                bt = cpool.tile([128, N], BF16, tag=f"BT{h}")
                if h == 0:
                    nc.vector.tensor_scalar_add(bt[:], pb[:], bc[:, 0:1])
                else:
                    nc.scalar.add(bt[:], pb[:], bc[:, 1:2])
                BT[h] = bt

            def fused_B(eng, S, p, h, w1, w2, cB):
                """One op for block-B half h over all four v rows."""
                sfull = S[:]
                out = bass.AP(sfull.tensor, sfull.offset + 2 * w1 + h * w2,
                              [sfull.ap[0], [SEG, RPB], [1, w2]])
                btf = BT[h][:]
                in0 = bass.AP(btf.tensor, btf.offset + cB,
                              [btf.ap[0], [0, RPB], [1, w2]])
                rsf = RS16[h][:]
                in1 = bass.AP(rsf.tensor, rsf.offset + RPB * (15 - p),
                              [rsf.ap[0], [1, RPB], [0, w2]])
                eng.tensor_tensor(out=out, in0=in0, in1=in1, op=ADD)

            # ---- main loop: one slab per block pair ----------------------
            for p in range(8):
                w1 = N - 32 * p          # block p rect width (cols 32p..512)
                w2 = 32 + 32 * p         # block 15-p width
                cA = 32 * p              # B'T col offset for block p
                cB = N - w2              # for block 15-p
                S = spool.tile([128, SLABW], BF16, tag="s", name=f"s{p}")
                for v in range(RPB):
                    off = SEG * v
                    mA = RPB * p + v
                    mB = RPB * (15 - p) + v
                    sA0 = S[:, off:off + w1]
                    sA1 = S[:, off + w1:off + 2 * w1]
                    # h0 wide segment: DVE tensor_scalar (4x uop)
                    nc.vector.tensor_scalar_add(
                        sA0, BT[0][:, cA:N], RS[0][:, mA:mA + 1])
                    # h1 wide segment: ACT for wide p, DVE for narrow p
                    if p <= 3:
                        nc.scalar.add(sA1, BT[1][:, cA:N], RS[1][:, mA:mA + 1])
                    else:
                        nc.vector.tensor_scalar_add(
                            sA1, BT[1][:, cA:N], RS[1][:, mA:mA + 1])
                    if p >= 6:
                        # wide block-B segments: individual ACT ops
                        for h in range(2):
                            sB = S[:, off + 2 * w1 + h * w2:
                                   off + 2 * w1 + (h + 1) * w2]
                            nc.scalar.add(sB, BT[h][:, cB:N],
                                          RS[h][:, mB:mB + 1])
                # narrow block-B segments: fused over v
                if p <= 3:
                    fused_B(nc.gpsimd, S, p, 0, w1, w2, cB)
                    fused_B(nc.gpsimd, S, p, 1, w1, w2, cB)
                elif p <= 5:
                    fused_B(nc.vector, S, p, 0, w1, w2, cB)
                    fused_B(nc.vector, S, p, 1, w1, w2, cB)
                nc.sync.dma_start(out=slab[p], in_=S[:])

    nc.compile()
    return nc


def _program() -> bass.Bass:
    global _PROGRAM
    if _PROGRAM is None:
        _PROGRAM = _build_program()
    return _PROGRAM


# --------------------------------------------------------------------------
# host entry point
# --------------------------------------------------------------------------

def build_in_maps(x, W, b):
    x = np.asarray(x, np.float32)
    W = np.asarray(W, np.float32)
    b = np.asarray(b, np.float32)
    w12 = np.concatenate(
        [np.ascontiguousarray(W[:, :NIN].T),
         np.ascontiguousarray(W[:, NIN:].T)], axis=1)  # [in, 512]
    shared = {
        "xt": np.ascontiguousarray(x.T).astype(BF16NP),
        "w12t": w12.astype(BF16NP),
        "bcol": np.ascontiguousarray(
            np.stack([b[0:128], b[128:256]], axis=1)),
    }
    maps = []
    for k in range(NCORES):
        rows = _rows_for_core(k)
        xsel = np.ascontiguousarray(x[rows].T).astype(BF16NP)
        maps.append(dict(shared, xselt=xsel))
    return maps


def _assemble(results):
    """8 per-core slab dicts -> full [512, 512, 256] f32 output."""
    out = np.zeros((N, N, CH), np.float32)
    ar = np.arange(RPB)
    for k in range(NCORES):
        slab = np.asarray(results[k]["slab"]).astype(np.float32)
        for p in range(8):
            w1 = N - 32 * p
            w2 = 32 + 32 * p
            sp = slab[p].reshape(128, RPB, SEG)
            rowsA = 32 * p + RPB * k + ar
            rowsB = 32 * (15 - p) + RPB * k + ar
            out[rowsA, 32 * p:N, 0:128] = \
                sp[:, :, 0:w1].transpose(1, 2, 0)
            out[rowsA, 32 * p:N, 128:256] = \
                sp[:, :, w1:2 * w1].transpose(1, 2, 0)
            out[rowsB, N - w2:N, 0:128] = \
                sp[:, :, 2 * w1:2 * w1 + w2].transpose(1, 2, 0)
            out[rowsB, N - w2:N, 128:256] = \
                sp[:, :, 2 * w1 + w2:SEG].transpose(1, 2, 0)
    # unshard: keep the strict upper triangle (sub-diagonal rect lanes are
    # garbage), mirror it across the diagonal; diag stays scatter-init 0.
    tril = np.tril_indices(N)
    out[tril] = 0.0
    return out + out.transpose(1, 0, 2)


def kernel(x, W, b):
    nc = _program()
    in_maps = build_in_maps(x, W, b)
    res = run_bass_kernel_spmd(nc, in_maps, core_ids=list(range(NCORES)))
    return _assemble(res.results)
